# revision 1
# baseline (speedup 1.0000x reference)
"""BertAttention (QKV + MHA + output proj + residual + LayerNorm) on 8 TRN2 cores.

Sharding: heads 2c,2c+1 -> core c (tensor-parallel attention); output
projection + LayerNorm token-sharded (512 flat tokens per core) after an
on-device AllToAll of the normalized per-head context.

Layout: activations kept transposed ([feature, token]) so every matmul
streams 512-token tiles at full PE speed.  Softmax runs in S^T ([key, query])
layout: exp on ACT (mask folded into the activation bias, 1/8 scale into the
activation scale), softmax denominators ride a ones-column augmented onto V
(the ctx matmul produces them for free), normalization via DVE fast
reciprocal + gpsimd partition_broadcast.  Head pair is packed into the PE
array (rows 0-63 / 64-127) for the K=64 score matmuls.
"""
import sys

sys.path.insert(0, "/opt/trn_rl_repo")

import numpy as np
import ml_dtypes

import concourse.bacc as bacc
import concourse.mybir as mybir
import concourse.tile as tile
from concourse.bass_utils import run_bass_kernel_spmd
from concourse.masks import make_identity

B, S, H = 2, 2048, 1024
NH, HD = 16, 64
W = 8                    # cores
T = B * S                # 4096 flat tokens
TOK = T // W             # 512 tokens owned per core
CPC = (NH // W) * HD     # 128 ctx channels per core (2 heads)
QT = 512                 # query tile (matmul free dim)
NQT = S // QT            # 4 query tiles per batch
NKC = S // 128           # 16 key chunks per batch
NK = H // 128            # 8 contraction chunks for the projections

F32 = mybir.dt.float32
BF16 = mybir.dt.bfloat16
BF = ml_dtypes.bfloat16

_NC_CACHE = {}


PHASE_MARKS = []


def _mark(nc, name):
    PHASE_MARKS.append((name, int(nc.next_id())))


def build_nc(no_collective=False, reps=1, ctx_bufs=2, qkv_bufs=2, pack=True, xt_bufs=8, tail_split=1, es_bufs=5, vs_bufs=2, st_bufs=4, z_bufs=2, qkv_first=False):
    PHASE_MARKS.clear()
    nc = bacc.Bacc(None)

    xT = nc.dram_tensor("xT", [H, T], BF16, kind="ExternalInput")
    wq = nc.dram_tensor("wq", [H, CPC], BF16, kind="ExternalInput")
    wk = nc.dram_tensor("wk", [H, CPC], BF16, kind="ExternalInput")
    wv = nc.dram_tensor("wv", [H, CPC], BF16, kind="ExternalInput")
    bq = nc.dram_tensor("bq", [CPC], F32, kind="ExternalInput")
    bk = nc.dram_tensor("bk", [CPC], F32, kind="ExternalInput")
    bv = nc.dram_tensor("bv", [CPC], F32, kind="ExternalInput")
    wo = nc.dram_tensor("wo", [H, H], BF16, kind="ExternalInput")
    bo = nc.dram_tensor("bo", [H], F32, kind="ExternalInput")
    gamma = nc.dram_tensor("gamma", [H], F32, kind="ExternalInput")
    beta = nc.dram_tensor("beta", [H], F32, kind="ExternalInput")
    hT = nc.dram_tensor("hT", [H, TOK], F32, kind="ExternalInput")
    maskT = nc.dram_tensor("maskT", [B, S], F32, kind="ExternalInput")
    y = nc.dram_tensor("y", [H, TOK], F32, kind="ExternalOutput")

    from contextlib import ExitStack
    with tile.TileContext(nc) as tc, ExitStack() as _stk:
        if True:
            constp = _stk.enter_context(tc.tile_pool(name="const", bufs=1))
            wpool = _stk.enter_context(tc.tile_pool(name="weights", bufs=1))
            xtp = _stk.enter_context(tc.tile_pool(name="xt", bufs=xt_bufs))
            qkp = _stk.enter_context(tc.tile_pool(name="qk", bufs=2))
            vsp = _stk.enter_context(tc.tile_pool(name="vstage", bufs=vs_bufs))
            vap = _stk.enter_context(tc.tile_pool(name="vaug", bufs=32))
            expp = _stk.enter_context(tc.tile_pool(name="exps", bufs=es_bufs))
            zp = _stk.enter_context(tc.tile_pool(name="znorm", bufs=z_bufs))
            stp = _stk.enter_context(tc.tile_pool(name="stage", bufs=st_bufs))
            dramp = _stk.enter_context(tc.tile_pool(name="dram", bufs=1, space="DRAM"))
            lnp = _stk.enter_context(tc.tile_pool(name="ln", bufs=2))
            xtl = _stk.enter_context(tc.tile_pool(name="xtile", bufs=NK))
            cop = _stk.enter_context(tc.tile_pool(name="ctxown", bufs=1))
            # ---- constants ----
            ident = constp.tile([128, 128], BF16)
            make_identity(nc, ident[:])
            ones128 = constp.tile([128, 1], BF16)
            nc.vector.memset(ones128[:], 1.0)
            a2a_in = dramp.tile([W, CPC, TOK], BF16)
            a2a_out = dramp.tile([W, CPC, TOK], BF16)

            for rep in range(reps):
                # ---- small inputs / weights (reloaded per rep) ----
                biases = constp.tile([128, 3], F32, tag="biases", bufs=min(reps, 2))
                nc.sync.dma_start(out=biases[:, 0:1], in_=bq[:].unsqueeze(1))
                nc.sync.dma_start(out=biases[:, 1:2], in_=bk[:].unsqueeze(1))
                nc.sync.dma_start(out=biases[:, 2:3], in_=bv[:].unsqueeze(1))
                mask_sb = constp.tile([128, B, NKC], F32, tag="mask", bufs=min(reps, 2))
                nc.sync.dma_start(
                    out=mask_sb[:, :, :],
                    in_=maskT.rearrange("b (j p) -> p b j", p=128),
                )
                wq_sb = wpool.tile([128, NK, CPC], BF16, tag="wq", bufs=1)
                wk_sb = wpool.tile([128, NK, CPC], BF16, tag="wk", bufs=1)
                wv_sb = wpool.tile([128, NK, CPC], BF16, tag="wv", bufs=1)
                for w_dram, w_sb in ((wq, wq_sb), (wk, wk_sb), (wv, wv_sb)):
                    wre = w_dram.rearrange("(c p) m -> p c m", p=128)
                    nc.sync.dma_start(out=w_sb[:, 0:NK // 2, :], in_=wre[:, 0:NK // 2, :])
                    nc.sync.dma_start(out=w_sb[:, NK // 2:NK, :], in_=wre[:, NK // 2:NK, :])
                wo_sb = wpool.tile([128, NK, H], BF16, tag="wo", bufs=1)
                hT_sb = wpool.tile([128, NK, TOK], F32, tag="hT", bufs=1)
                obg = constp.tile([128, NK, 3], F32, tag="obg", bufs=min(reps, 2))

                with ExitStack() as _ps_stk:
                    qkv_ps = _ps_stk.enter_context(tc.tile_pool(name=f"qkv_ps{rep}", bufs=qkv_bufs, space="PSUM"))
                    sc_ps = _ps_stk.enter_context(tc.tile_pool(name=f"sc_ps{rep}", bufs=2, space="PSUM"))
                    ctx_ps = _ps_stk.enter_context(tc.tile_pool(name=f"ctx_ps{rep}", bufs=ctx_bufs, space="PSUM"))
                    batch_qkv = {}
                    for b in range(B):
                        _mark(nc, f"qkv_b{b}")
                        # -------- QKV projections --------
                        xt_tiles = []
                        for k in range(NK):
                            xt_t = xtp.tile([128, S], BF16, tag="xt")
                            nc.sync.dma_start(
                                out=xt_t[:, 0:S // 2],
                                in_=xT[k * 128:(k + 1) * 128,
                                       b * S:b * S + S // 2],
                            )
                            nc.sync.dma_start(
                                out=xt_t[:, S // 2:S],
                                in_=xT[k * 128:(k + 1) * 128,
                                       b * S + S // 2:(b + 1) * S],
                            )
                            xt_tiles.append(xt_t)

                        qTt = qkp.tile([128, S], BF16, tag="qT")
                        kTt = qkp.tile([128, S], BF16, tag="kT")
                        vaug_tiles = []
                        for t in range(NQT):
                            tsl = slice(t * QT, (t + 1) * QT)
                            for w_sb, bcol, dstT in (
                                (wq_sb, 0, qTt), (wk_sb, 1, kTt), (wv_sb, 2, None)
                            ):
                                ps = qkv_ps.tile([128, QT], F32, tag="qkv")
                                for k in range(NK):
                                    nc.tensor.matmul(
                                        ps[:, :],
                                        w_sb[:, k, :],
                                        xt_tiles[k][:, tsl],
                                        start=(k == 0),
                                        stop=(k == NK - 1),
                                    )
                                if dstT is not None:
                                    nc.vector.tensor_scalar_add(
                                        dstT[:, tsl], ps[:, :],
                                        biases[:, bcol:bcol + 1],
                                    )
                                else:
                                    vst = vsp.tile([128, QT], BF16, tag="vst")
                                    nc.vector.tensor_scalar_add(
                                        vst[:, :], ps[:, :],
                                        biases[:, bcol:bcol + 1],
                                    )
                                    for s4 in range(QT // 128):
                                        vps = qkv_ps.tile(
                                            [128, 128], BF16, tag="qkv"
                                        )
                                        nc.tensor.transpose(
                                            vps[:, :],
                                            vst[:, s4 * 128:(s4 + 1) * 128],
                                            ident[:, :],
                                        )
                                        va = vap.tile([128, 130], BF16, tag="vaug")
                                        nc.vector.memset(va[:, 64:65], 1.0)
                                        nc.vector.memset(va[:, 129:130], 1.0)
                                        nc.vector.tensor_copy(
                                            va[:, 0:64], vps[:, 0:64]
                                        )
                                        nc.vector.tensor_copy(
                                            va[:, 65:129], vps[:, 64:128]
                                        )
                                        vaug_tiles.append(va)

                        batch_qkv[b] = (qTt, kTt, vaug_tiles)
                        if qkv_first and b < B - 1:
                            continue
                        for ab in ((b,) if not qkv_first else range(B)):
                          qTt, kTt, vaug_tiles = batch_qkv[ab]
                          _mark(nc, f"attn_b{ab}")
                          if True:
                            # -------- attention --------
                            for t in range(NQT):
                              tsl = slice(t * QT, (t + 1) * QT)
                              cpA = ctx_ps.tile([128, QT], F32, tag="ctx")
                              cpB = ctx_ps.tile([128, QT], F32, tag="ctx")
                              for j in range(NKC):
                                  jsl = slice(j * 128, (j + 1) * 128)
                                  sp = sc_ps.tile([128, 2, QT], F32, tag="sc")
                                  nc.tensor.matmul(
                                      sp[:, 0, :], kTt[0:64, jsl], qTt[0:64, tsl],
                                      start=True, stop=True,
                                      tile_position=(0, 0) if pack else None,
                                  )
                                  nc.tensor.matmul(
                                      sp[:, 1, :], kTt[64:128, jsl], qTt[64:128, tsl],
                                      start=True, stop=True,
                                      tile_position=(64, 0) if pack else None,
                                  )
                                  es = expp.tile([128, 2, QT], BF16, tag="es")
                                  nc.scalar.activation(
                                      es[:, :, :], sp[:, :, :],
                                      mybir.ActivationFunctionType.Exp,
                                      bias=mask_sb[:, ab, j:j + 1], scale=0.125,
                                  )
                                  va = vaug_tiles[j]
                                  nc.tensor.matmul(
                                      cpA[0:65, :], va[:, 0:65], es[:, 0, :],
                                      start=(j == 0), stop=(j == NKC - 1),
                                  )
                                  nc.tensor.matmul(
                                      cpB[0:65, :], va[:, 65:130], es[:, 1, :],
                                      start=(j == 0), stop=(j == NKC - 1),
                                  )
                              # evacuate ctx psum fast (releases the banks),
                              # then normalize from SBUF off the critical path
                              caS = zp.tile([64, QT], BF16, tag="caS", bufs=3)
                              cbS = zp.tile([64, QT], BF16, tag="cbS", bufs=3)
                              nc.vector.tensor_copy(caS[:, :], cpA[0:64, :])
                              nc.vector.tensor_copy(cbS[:, :], cpB[0:64, :])
                              zrA = zp.tile([1, QT], F32, tag="zr")
                              zrB = zp.tile([1, QT], F32, tag="zr")
                              nc.vector.tensor_copy(zrA[:, :], cpA[64:65, :])
                              nc.vector.tensor_copy(zrB[:, :], cpB[64:65, :])
                              zA = zp.tile([1, QT], F32, tag="z")
                              zB = zp.tile([1, QT], F32, tag="z")
                              nc.vector.reciprocal_approx_fast(zA[:, :], zrA[:, :])
                              nc.vector.reciprocal_approx_fast(zB[:, :], zrB[:, :])
                              rbA = zp.tile([64, QT], F32, tag="rb")
                              rbB = zp.tile([64, QT], F32, tag="rb")
                              nc.gpsimd.partition_broadcast(
                                  rbA[:, :], zA[:, :], channels=64
                              )
                              nc.gpsimd.partition_broadcast(
                                  rbB[:, :], zB[:, :], channels=64
                              )
                              stA = stp.tile([64, QT], BF16, tag="st")
                              stB = stp.tile([64, QT], BF16, tag="st")
                              nc.vector.tensor_mul(stA[:, :], caS[:, :], rbA[:, :])
                              nc.vector.tensor_mul(stB[:, :], cbS[:, :], rbB[:, :])
                              d = ab * NQT + t
                              nc.gpsimd.dma_start(
                                  out=a2a_in[d, 0:64, :], in_=stA[:, :]
                              )
                              nc.gpsimd.dma_start(
                                  out=a2a_in[d, 64:128, :], in_=stB[:, :]
                              )

                nc.sync.dma_start(
                    out=wo_sb[:, :, :], in_=wo.rearrange("(c p) m -> p c m", p=128)
                )
                nc.sync.dma_start(
                    out=hT_sb[:, :, :], in_=hT.rearrange("(c p) m -> p c m", p=128)
                )
                nc.sync.dma_start(
                    out=obg[:, :, 0], in_=bo.rearrange("(c p) -> p c", p=128)
                )
                nc.sync.dma_start(
                    out=obg[:, :, 1], in_=gamma.rearrange("(c p) -> p c", p=128)
                )
                nc.sync.dma_start(
                    out=obg[:, :, 2], in_=beta.rearrange("(c p) -> p c", p=128)
                )
                _mark(nc, "a2a")
                # ---- all-to-all: heads -> tokens resharding ----
                if no_collective:
                    for i in range(W):
                        nc.sync.dma_start(out=a2a_out[i, :, :], in_=a2a_in[i, :, :])
                else:
                    nc.gpsimd.collective_compute(
                        "AllToAll",
                        mybir.AluOpType.bypass,
                        replica_groups=[list(range(W))],
                        ins=[a2a_in[:, :, :].opt()],
                        outs=[a2a_out[:, :, :].opt()],
                    )

                # ---- output projection + residual + LayerNorm ----
                _mark(nc, "tail")
                with ExitStack() as _op_stk:
                    op_ps = _op_stk.enter_context(tc.tile_pool(name=f"op_ps{rep}", bufs=2, space="PSUM"))
                    mom_ps = _op_stk.enter_context(tc.tile_pool(name=f"mom_ps{rep}", bufs=2, space="PSUM"))
                    ctx_own = cop.tile([128, NK, TOK], BF16, tag="ctxown")
                    for i in range(W):
                        nc.sync.dma_start(
                            out=ctx_own[:, i, :], in_=a2a_out[i, :, :]
                        )
                    TS = TOK // tail_split
                    for h in range(tail_split):
                        hs = slice(h * TS, (h + 1) * TS)
                        mom1 = mom_ps.tile([1, TS], F32, tag="mom")
                        mom2 = mom_ps.tile([1, TS], F32, tag="mom")
                        xts = []
                        for o in range(NK):
                            ps = op_ps.tile([128, TS], F32, tag="op")
                            for k in range(NK):
                                nc.tensor.matmul(
                                    ps[:, :],
                                    wo_sb[:, k, o * 128:(o + 1) * 128],
                                    ctx_own[:, k, hs],
                                    start=(k == 0),
                                    stop=(k == NK - 1),
                                )
                            xt_o = xtl.tile([128, TS], F32, tag="xt_o")
                            nc.vector.scalar_tensor_tensor(
                                xt_o[:, :], ps[:, :], obg[:, o, 0:1],
                                hT_sb[:, o, hs],
                                op0=mybir.AluOpType.add, op1=mybir.AluOpType.add,
                            )
                            xts.append(xt_o)
                            xt_bf = stp.tile([128, TS], BF16, tag="xtbf", bufs=2)
                            nc.vector.tensor_copy(xt_bf[:, :], xt_o[:, :])
                            sq = stp.tile([128, TS], BF16, tag="sq", bufs=2)
                            nc.vector.tensor_mul(sq[:, :], xt_o[:, :], xt_o[:, :])
                            nc.tensor.matmul(
                                mom1[:, :], ones128[:, :], xt_bf[:, :],
                                start=(o == 0), stop=(o == NK - 1),
                                skip_group_check=True,
                            )
                            nc.tensor.matmul(
                                mom2[:, :], ones128[:, :], sq[:, :],
                                start=(o == 0), stop=(o == NK - 1),
                                skip_group_check=True,
                            )
                        # mean / var -> rstd
                        muZ = lnp.tile([1, TS], F32, tag="muz", bufs=2)
                        m2Z = lnp.tile([1, TS], F32, tag="m2z", bufs=2)
                        nc.vector.tensor_scalar_mul(muZ[:, :], mom1[:, :], 1.0 / H)
                        nc.vector.tensor_scalar_mul(m2Z[:, :], mom2[:, :], 1.0 / H)
                        mu_b = lnp.tile([128, TS], F32, tag="mub", bufs=2)
                        m2_b = lnp.tile([128, TS], F32, tag="m2b", bufs=2)
                        nc.gpsimd.partition_broadcast(mu_b[:, :], muZ[:, :], channels=128)
                        nc.gpsimd.partition_broadcast(m2_b[:, :], m2Z[:, :], channels=128)
                        musq = lnp.tile([128, TS], F32, tag="musq", bufs=2)
                        nc.vector.tensor_mul(musq[:, :], mu_b[:, :], mu_b[:, :])
                        vare = lnp.tile([128, TS], F32, tag="vare", bufs=2)
                        nc.vector.tensor_sub(vare[:, :], m2_b[:, :], musq[:, :])
                        vare2 = lnp.tile([128, TS], F32, tag="vare2", bufs=2)
                        nc.vector.tensor_scalar_add(vare2[:, :], vare[:, :], 1e-12)
                        rvar = lnp.tile([128, TS], F32, tag="rvar", bufs=2)
                        nc.vector.reciprocal_approx_fast(rvar[:, :], vare2[:, :])
                        rstd = lnp.tile([128, TS], F32, tag="rstd", bufs=2)
                        nc.scalar.activation(
                            rstd[:, :], rvar[:, :],
                            mybir.ActivationFunctionType.Sqrt,
                        )
                        for o in range(NK):
                            dcen = stp.tile([128, TS], F32, tag="dcen", bufs=2)
                            nc.vector.tensor_sub(dcen[:, :], xts[o][:, :], mu_b[:, :])
                            en = stp.tile([128, TS], F32, tag="en", bufs=2)
                            nc.vector.tensor_mul(en[:, :], dcen[:, :], rstd[:, :])
                            outt = stp.tile([128, TS], F32, tag="outt", bufs=2)
                            nc.vector.tensor_scalar(
                                outt[:, :], en[:, :],
                                obg[:, o, 1:2], obg[:, o, 2:3],
                                op0=mybir.AluOpType.mult, op1=mybir.AluOpType.add,
                            )
                            nc.sync.dma_start(
                                out=y[o * 128:(o + 1) * 128, hs], in_=outt[:, :]
                            )

    _mark(nc, "end")
    nc.compile()
    return nc


def get_nc():
    if "nc" not in _NC_CACHE:
        _NC_CACHE["nc"] = build_nc()
    return _NC_CACHE["nc"]


def prepare_in_maps(inputs):
    hidden = np.asarray(inputs["hidden_states"], dtype=np.float32)
    mask = np.asarray(inputs["attention_mask"], dtype=np.float32)
    Wq = np.asarray(inputs["Wq"], dtype=np.float32)
    Wk = np.asarray(inputs["Wk"], dtype=np.float32)
    Wv = np.asarray(inputs["Wv"], dtype=np.float32)
    Wo = np.asarray(inputs["Wo"], dtype=np.float32)
    bq = np.asarray(inputs["bq"], dtype=np.float32)
    bk = np.asarray(inputs["bk"], dtype=np.float32)
    bv = np.asarray(inputs["bv"], dtype=np.float32)
    bo = np.asarray(inputs["bo"], dtype=np.float32)
    gamma = np.asarray(inputs["ln_gamma"], dtype=np.float32)
    beta = np.asarray(inputs["ln_beta"], dtype=np.float32)

    X = hidden.reshape(T, H)
    xT = np.ascontiguousarray(X.T)                      # [H, T] f32
    xT_bf = xT.astype(BF)
    maskT_np = np.ascontiguousarray(mask.reshape(B, S))
    wo_bf = Wo.astype(BF)

    in_maps = []
    for c in range(W):
        csl = slice(CPC * c, CPC * (c + 1))
        tsl = slice(TOK * c, TOK * (c + 1))
        in_maps.append({
            "xT": xT_bf,
            "wq": np.ascontiguousarray(Wq[:, csl]).astype(BF),
            "wk": np.ascontiguousarray(Wk[:, csl]).astype(BF),
            "wv": np.ascontiguousarray(Wv[:, csl]).astype(BF),
            "bq": np.ascontiguousarray(bq[csl]),
            "bk": np.ascontiguousarray(bk[csl]),
            "bv": np.ascontiguousarray(bv[csl]),
            "wo": wo_bf,
            "bo": bo,
            "gamma": gamma,
            "beta": beta,
            "hT": np.ascontiguousarray(xT[:, tsl]),
            "maskT": maskT_np,
        })
    return in_maps


def kernel(**inputs):
    in_maps = prepare_in_maps(inputs)
    nc = get_nc()
    res = run_bass_kernel_spmd(nc, in_maps, core_ids=list(range(W)))
    out_flat = np.empty((T, H), dtype=np.float32)
    for c in range(W):
        out_flat[TOK * c:TOK * (c + 1), :] = res.results[c]["y"].T
    return out_flat.reshape(B, S, H)

